# revision 3
# baseline (speedup 1.0000x reference)
"""Causal self-attention with RoPE for Trainium2, sharded over 8 NeuronCores.

Sharding (Megatron-style, per the problem's hint):
  8 cores = 4 batches x 2 head-groups (8 of 16 heads each).
  Each core: QKV column-slice projections [1024,512], RoPE, causal attention
  for its 8 heads, and a row-slice output projection producing a partial
  [2048,1024]. Host sums the two partials per batch and adds bo.

Per-core device kernel (Tile framework), all matmuls bf16:
  Stage A: Q/K/V in [t,c] layout (lhsT = x^T chunks), RoPE on DVE via
           even/odd strided views, PE-transpose q,k into [c,t] layout;
           V stored with an appended ones column (V_aug) per head.
  Stage B: scores computed transposed S^T[j,q] = k^T.T @ q^T (K=64; two
           heads run concurrently in row-groups 0/64), exp on ACT with
           the 1/sqrt(64) scale folded in, causal masks via DVE multiply,
           AV matmul with M=65 yielding both Y^T and the softmax
           denominator in one accumulation chain; normalization via
           reciprocal + K=1 ones-matmul partition broadcast.
  Stage C: output projection from the [c,t]-layout Y^T (K=128 chunks),
           PSUM DMA'd straight to DRAM.

No flash-attention running max is needed: scores are ~N(0, 0.17) here and
exp cannot overflow; softmax(x) == softmax(x - max) exactly.
"""
import sys

if "/opt/trn_rl_repo" not in sys.path:
    sys.path.insert(0, "/opt/trn_rl_repo")

from contextlib import ExitStack

import numpy as np
import ml_dtypes

import concourse.bass as bass
import concourse.mybir as mybir
import concourse.tile as tile
from concourse import bacc
from concourse._compat import with_exitstack
from concourse.masks import make_identity

bf16 = ml_dtypes.bfloat16

N_HEAD = 16
ROPE_BASE = 10000.0
B_FULL, T_FULL, C_FULL = 4, 2048, 1024
HD = 64
N_CORES = 8
QCW = 512  # query-chunk width
JBW = 128  # key-block width


def build_core_program(T=T_FULL, HL=8, C=C_FULL, has_bias=False):
    """Build the per-core Bass program. Returns (nc, names) where names
    lists the DRAM input tensor names."""
    CL = HL * HD            # local c width (512 full)
    NTB = T // 128          # t-blocks
    NQC = T // QCW          # query chunks
    NCH = CL // 128         # head-pairs (c-chunks of 128)
    KCH = C // 128          # contraction chunks of the input dim
    NEH = C // 512          # output halves

    f32 = mybir.dt.float32
    b16 = mybir.dt.bfloat16

    nc = bacc.Bacc("TRN2", target_bir_lowering=False, debug=False,
                   enable_asserts=False)

    xT = nc.dram_tensor("xT", [C, T], b16, kind="ExternalInput").ap()
    wq = nc.dram_tensor("wq", [C, CL], b16, kind="ExternalInput").ap()
    wk = nc.dram_tensor("wk", [C, CL], b16, kind="ExternalInput").ap()
    wv = nc.dram_tensor("wv", [C, CL], b16, kind="ExternalInput").ap()
    wo = nc.dram_tensor("wo", [CL, C], b16, kind="ExternalInput").ap()
    cosd = nc.dram_tensor("cosw", [T, 32], f32, kind="ExternalInput").ap()
    sind = nc.dram_tensor("sinw", [T, 32], f32, kind="ExternalInput").ap()
    maskd = nc.dram_tensor("masks", [4, JBW, QCW], b16, kind="ExternalInput").ap()
    o = nc.dram_tensor("o", [T, C], f32, kind="ExternalOutput").ap()
    names = ["xT", "wq", "wk", "wv", "wo", "cosw", "sinw", "masks"]
    if has_bias:
        bqr = nc.dram_tensor("bqr", [1, CL], b16, kind="ExternalInput").ap()
        bkr = nc.dram_tensor("bkr", [1, CL], b16, kind="ExternalInput").ap()
        bvr = nc.dram_tensor("bvr", [1, CL], b16, kind="ExternalInput").ap()
        names += ["bqr", "bkr", "bvr"]

    with tile.TileContext(nc) as tc:
        _body(ExitStack(), tc, locals())
    nc.compile()
    return nc, names


def _body(ctx, tc, env):
    with ctx:
        _body_inner(ctx, tc, env)


def _body_inner(ctx, tc, env):
    nc = tc.nc
    f32 = mybir.dt.float32
    b16 = mybir.dt.bfloat16
    T, HL, C = env["T"], env["HL"], env["C"]
    CL, NTB, NQC, NCH, KCH, NEH = (env["CL"], env["NTB"], env["NQC"],
                                   env["NCH"], env["KCH"], env["NEH"])
    has_bias = env["has_bias"]
    xT, wq, wk, wv, wo = env["xT"], env["wq"], env["wk"], env["wv"], env["wo"]
    cosd, sind, maskd, o = env["cosd"], env["sind"], env["maskd"], env["o"]

    const = ctx.enter_context(tc.tile_pool(name="const", bufs=1))
    persist = ctx.enter_context(tc.tile_pool(name="persist", bufs=1))

    # ---- constants / weights into SBUF
    xT_sb = const.tile([128, KCH, T], b16)
    nc.sync.dma_start(out=xT_sb, in_=xT.rearrange("(kc p) t -> p kc t", p=128))
    wq_sb = const.tile([128, KCH, CL], b16)
    nc.sync.dma_start(out=wq_sb, in_=wq.rearrange("(kc p) c -> p kc c", p=128))
    wk_sb = const.tile([128, KCH, CL], b16)
    nc.sync.dma_start(out=wk_sb, in_=wk.rearrange("(kc p) c -> p kc c", p=128))
    wv_sb = const.tile([128, KCH, CL], b16)
    nc.sync.dma_start(out=wv_sb, in_=wv.rearrange("(kc p) c -> p kc c", p=128))
    wo_sb = const.tile([128, NCH, C], b16)
    nc.sync.dma_start(out=wo_sb, in_=wo.rearrange("(cc p) e -> p cc e", p=128))
    cos_sb = const.tile([128, NTB, 32], f32)
    nc.sync.dma_start(out=cos_sb, in_=cosd.rearrange("(n p) d -> p n d", p=128))
    sin_sb = const.tile([128, NTB, 32], f32)
    nc.sync.dma_start(out=sin_sb, in_=sind.rearrange("(n p) d -> p n d", p=128))
    mask_sb = const.tile([128, 4, QCW], b16)
    nc.sync.dma_start(out=mask_sb, in_=maskd.rearrange("m p q -> p m q"))
    ident = const.tile([128, 128], b16)
    make_identity(nc, ident)
    ones_sb = const.tile([1, 128], b16)
    nc.vector.memset(ones_sb, 1.0)
    if has_bias:
        bq_sb = const.tile([1, CL], b16, tag="bq")
        nc.sync.dma_start(out=bq_sb, in_=env["bqr"])
        bk_sb = const.tile([1, CL], b16, tag="bk")
        nc.sync.dma_start(out=bk_sb, in_=env["bkr"])
        bv_sb = const.tile([1, CL], b16, tag="bv")
        nc.sync.dma_start(out=bv_sb, in_=env["bvr"])
        brows = {"q": bq_sb, "k": bk_sb, "v": bv_sb}

    qT_sb = persist.tile([128, NCH, T], b16)
    kT_sb = persist.tile([128, NCH, T], b16)
    yT_sb = persist.tile([128, NCH, T], b16)
    vaug = persist.tile([128, NTB, HL, 65], b16)
    nc.vector.memset(vaug[:, :, :, 64:65], 1.0)

    def proj(ps, w_sb, tb, which):
        for kc in range(KCH):
            nc.tensor.matmul(ps, xT_sb[:, kc, tb * 128:(tb + 1) * 128],
                             w_sb[:, kc, :], start=(kc == 0),
                             stop=(kc == KCH - 1 and not has_bias))
        if has_bias:
            nc.tensor.matmul(ps, ones_sb, brows[which], start=False, stop=True)

    # ---------------- Stage A: projections + RoPE + transposes ----------
    with tc.tile_pool(name="psA", bufs=3, space="PSUM") as psA, \
         tc.tile_pool(name="psTr", bufs=2, space="PSUM") as psTr, \
         tc.tile_pool(name="workA", bufs=3) as workA, \
         tc.tile_pool(name="rope", bufs=2) as rope:
        for tb in range(NTB):
            def bchead(t):
                # [128, 32] -> [128, HL, 32] with a step-0 (broadcast) head dim
                return bass.AP(tensor=t.tensor, offset=t.offset,
                               ap=[t.ap[0], [0, HL], t.ap[1]])
            cosb = bchead(cos_sb[:, tb, :])
            sinb = bchead(sin_sb[:, tb, :])
            for which, w_sb, dstT in (("q", wq_sb, qT_sb), ("k", wk_sb, kT_sb)):
                ps = psA.tile([128, CL], f32, tag="psqk")
                proj(ps, w_sb, tb, which)
                x16 = workA.tile([128, CL], b16, tag="x16")
                nc.vector.tensor_copy(x16, ps)  # bf16 round (match reference)
                x4 = x16.rearrange("p (h i two) -> p h i two", two=2, i=32)
                ev, od = x4[:, :, :, 0], x4[:, :, :, 1]
                m1 = rope.tile([128, HL, 32], f32, tag="m1")
                m2 = rope.tile([128, HL, 32], f32, tag="m2")
                m3 = rope.tile([128, HL, 32], f32, tag="m3")
                m4 = rope.tile([128, HL, 32], f32, tag="m4")
                nc.vector.tensor_mul(m1, ev, cosb)
                nc.vector.tensor_mul(m2, od, sinb)
                nc.vector.tensor_mul(m3, ev, sinb)
                nc.vector.tensor_mul(m4, od, cosb)
                rot = workA.tile([128, CL], b16, tag="rot")
                r4 = rot.rearrange("p (h i two) -> p h i two", two=2, i=32)
                nc.vector.tensor_sub(r4[:, :, :, 0], m1, m2)
                nc.vector.tensor_add(r4[:, :, :, 1], m3, m4)
                pst = psTr.tile([128, CL], b16, tag="pst")
                for cb in range(NCH):
                    nc.tensor.transpose(pst[:, cb * 128:(cb + 1) * 128],
                                        rot[:, cb * 128:(cb + 1) * 128], ident)
                nc.vector.tensor_copy(
                    dstT[:, :, tb * 128:(tb + 1) * 128],
                    pst.rearrange("p (cb t) -> p cb t", cb=NCH))
            # V
            ps = psA.tile([128, CL], f32, tag="psv")
            proj(ps, wv_sb, tb, "v")
            nc.vector.tensor_copy(vaug[:, tb, :, 0:64],
                                  ps.rearrange("p (h d) -> p h d", d=64))

    # ---------------- Stage B: attention --------------------------------
    with tc.tile_pool(name="psS", bufs=2, space="PSUM") as psS, \
         tc.tile_pool(name="psAV", bufs=2, space="PSUM") as psAV, \
         tc.tile_pool(name="psBC", bufs=2, space="PSUM") as psBC, \
         tc.tile_pool(name="workB", bufs=3) as workB, \
         tc.tile_pool(name="small", bufs=2) as small:
        for qc in range(NQC):
            qs = qc * QCW
            njb = (qs + QCW) // JBW
            for g in range(NCH):
                avs = []
                for hh in range(2):
                    base = hh * 64
                    h = g * 2 + hh
                    ps_av = psAV.tile([65, QCW], f32, tag="av")
                    avs.append(ps_av)
                    for jbg in range(0, njb, 2):
                        ps_s = psS.tile([128, 2 * QCW], f32, tag="s")
                        for u in range(2):
                            jb = jbg + u
                            nc.tensor.matmul(
                                ps_s[:, u * QCW:(u + 1) * QCW],
                                kT_sb[base:base + 64, g,
                                      jb * JBW:(jb + 1) * JBW],
                                qT_sb[base:base + 64, g, qs:qs + QCW],
                                start=True, stop=True)
                        e = workB.tile([128, 2 * QCW], b16, tag="e")
                        nc.scalar.activation(
                            out=e, in_=ps_s,
                            func=mybir.ActivationFunctionType.Exp,
                            scale=float(1.0 / np.sqrt(HD)))
                        for u in range(2):
                            jb = jbg + u
                            if jb >= njb - 4:  # diagonal block
                                m = jb - (njb - 4)
                                nc.vector.tensor_mul(
                                    e[:, u * QCW:(u + 1) * QCW],
                                    e[:, u * QCW:(u + 1) * QCW],
                                    mask_sb[:, m, :])
                            nc.tensor.matmul(
                                ps_av, vaug[:, jb, h, :],
                                e[:, u * QCW:(u + 1) * QCW],
                                start=(jb == 0), stop=(jb == njb - 1))
                # normalize + store Y^T for both heads of the pair
                for hh in range(2):
                    base = hh * 64
                    ps_av = avs[hh]
                    rinv = small.tile([1, QCW], f32, tag="rinv")
                    nc.vector.reciprocal(rinv, ps_av[64:65, :])
                    rb16 = small.tile([1, QCW], b16, tag="rb16")
                    nc.vector.tensor_copy(rb16, rinv)
                    ps_bc = psBC.tile([64, QCW], f32, tag="bc")
                    nc.tensor.matmul(ps_bc, ones_sb[0:1, 0:64], rb16,
                                     start=True, stop=True)
                    rb = small.tile([64, QCW], f32, tag="rb")
                    nc.vector.tensor_copy(rb, ps_bc)
                    nc.vector.tensor_mul(
                        yT_sb[base:base + 64, g, qs:qs + QCW],
                        ps_av[0:64, :], rb)

    # ---------------- Stage C: output projection ------------------------
    with tc.tile_pool(name="psO", bufs=4, space="PSUM") as psO, \
         tc.tile_pool(name="workC", bufs=4) as workC:
        for tb in range(NTB):
            for eh in range(NEH):
                ps_o = psO.tile([128, 512], f32, tag="o")
                for cc in range(NCH):
                    nc.tensor.matmul(ps_o,
                                     yT_sb[:, cc, tb * 128:(tb + 1) * 128],
                                     wo_sb[:, cc, eh * 512:(eh + 1) * 512],
                                     start=(cc == 0), stop=(cc == NCH - 1))
                o_sb = workC.tile([128, 512], f32, tag="osb")
                nc.vector.tensor_copy(o_sb, ps_o)
                nc.sync.dma_start(
                    out=o[tb * 128:(tb + 1) * 128, eh * 512:(eh + 1) * 512],
                    in_=o_sb)


def make_host_aux(T=T_FULL):
    """cos/sin caches [T, 32] f32 and causal masks [4, 128, 512] bf16."""
    inv_freq = (1.0 / ROPE_BASE ** (np.arange(0, HD, 2, dtype=np.float32)
                                    / np.float32(HD))).astype(np.float32)
    pos = np.arange(T, dtype=np.float32)
    freqs = np.outer(pos, inv_freq).astype(np.float32)
    cos, sin = np.cos(freqs).astype(np.float32), np.sin(freqs).astype(np.float32)
    jf = np.arange(JBW)[:, None]
    qf = np.arange(QCW)[None, :]
    masks = np.stack([(qf >= m * JBW + jf) for m in range(4)]).astype(bf16)
    return cos, sin, masks


def make_in_maps(x, Wq, bq, Wk, bk, Wv, bv, Wo, T=T_FULL, HL=8):
    """Shard inputs for the 8 cores: core i = (batch i//2, head-group i%2)."""
    CL = HL * HD
    cos, sin, masks = make_host_aux(T)
    B = x.shape[0]
    n_groups = N_CORES // B
    has_bias = bool(np.any(bq) or np.any(bk) or np.any(bv))
    in_maps = []
    for core in range(N_CORES):
        b, g = divmod(core, n_groups)
        cols = slice(g * CL, (g + 1) * CL)
        m = {
            "xT": np.ascontiguousarray(x[b].astype(bf16).T),
            "wq": np.ascontiguousarray(Wq[:, cols].astype(bf16)),
            "wk": np.ascontiguousarray(Wk[:, cols].astype(bf16)),
            "wv": np.ascontiguousarray(Wv[:, cols].astype(bf16)),
            "wo": np.ascontiguousarray(Wo[cols, :].astype(bf16)),
            "cosw": cos, "sinw": sin, "masks": masks,
        }
        if has_bias:
            m["bqr"] = bq[None, cols].astype(bf16)
            m["bkr"] = bk[None, cols].astype(bf16)
            m["bvr"] = bv[None, cols].astype(bf16)
        in_maps.append(m)
    return in_maps, has_bias


_CACHE = {}


def kernel(x, Wq, bq, Wk, bk, Wv, bv, Wo, bo):
    x = np.asarray(x, np.float32)
    B, T, C = x.shape
    assert (B, T, C) == (B_FULL, T_FULL, C_FULL), (B, T, C)
    in_maps, has_bias = make_in_maps(x, Wq, bq, Wk, bk, Wv, bv, Wo)
    key = ("full", has_bias)
    if key not in _CACHE:
        _CACHE[key] = build_core_program(T=T_FULL, HL=8, C=C_FULL,
                                         has_bias=has_bias)
    nc, _names = _CACHE[key]
    from concourse.bass_utils import run_bass_kernel_spmd
    res = run_bass_kernel_spmd(nc, in_maps, core_ids=list(range(N_CORES)),
                               trace=False)
    bo32 = np.asarray(bo, np.float32)
    out = np.empty((B, T, C), np.float32)
    n_groups = N_CORES // B
    for b in range(B):
        acc = res.results[b * n_groups]["o"].astype(np.float32)
        for g in range(1, n_groups):
            acc = acc + res.results[b * n_groups + g]["o"]
        out[b] = acc + bo32[None, :]
    return out
